# revision 7
# baseline (speedup 1.0000x reference)
"""Trainium2 Bass kernel for nn_Decoder (GNN message passing decoder).

Strategy:
  - Host sorts graphs by num_nodes into 9 uniform blocks -> no masks on device.
  - Data parallel over 8 cores (round-robin within each block so every core
    gets an identical block structure -> one SPMD program).
  - Device layout is feature-major: [features on partitions, cols = graphs/
    nodes/pairs]. All matmuls run on the PE in fp32r (1 cyc/row).
  - GIN layer 0 is folded: x0 = [g ; lap_row] with lap rows constant per
    block, so z0 = (1+n)*(g @ Wa[:64]) + C[n][i] with C host-precomputed.
  - Aggregation s = h + sum_j h_j uses linearity: z = h@Wa + (sum h)@Wa.
  - GraphNorm via grouped DVE reduces + scalar_tensor_tensor, rsqrt via
    ACT Sqrt + reciprocal_approx_accurate.
  - Edge MLP computes only the n(n+1)/2 unique pairs per graph from
    u = 0.5*(h@W1e + b1e): z1(i,j) = u_i + u_j. Host mirrors to (j,i).
  - Final edge/node projections are DMA'd straight out of PSUM; host adds
    the last bias during scatter.
"""

import os
import sys
from contextlib import ExitStack

import numpy as np

for _p in ("/opt/trn_rl_repo", os.path.expanduser("~/.axon_site/_ro/trn_rl_repo")):
    if os.path.isdir(_p) and _p not in sys.path:
        sys.path.append(_p)

import concourse.bacc as bacc
import concourse.bass as bass
import concourse.tile as tile
from concourse import mybir
from concourse.bass_utils import run_bass_kernel_spmd

F32 = mybir.dt.float32
F32R = mybir.dt.float32r
AF = mybir.ActivationFunctionType
ALU = mybir.AluOpType
AX = mybir.AxisListType

NCORES = 8
MAXN = 9
NEG = 0.01
EPS = 1e-5
PSUM_COLS = 512


def _np(x):
    return np.asarray(x, dtype=np.float32)


class _Packer:
    """Packs [K, M] constants into one [128, X] fp32 array; returns slices."""

    def __init__(self):
        self.cols = 0
        self.items = []

    def add(self, arr, pad_m=None):
        arr = _np(arr)
        assert arr.ndim == 2 and arr.shape[0] <= 128
        if pad_m is not None and arr.shape[1] < pad_m:
            arr = np.pad(arr, ((0, 0), (0, pad_m - arr.shape[1])))
        off = self.cols
        self.cols += arr.shape[1]
        self.items.append((off, arr))
        return (off, arr.shape[0], arr.shape[1])

    def materialize(self):
        out = np.zeros((128, max(self.cols, 1)), np.float32)
        for off, arr in self.items:
            out[: arr.shape[0], off : off + arr.shape[1]] = arr
        return out


def _prep_host(params, lap_table, num_nodes_arr):
    """All host-side preprocessing that depends only on params/lap/num_nodes."""
    p = params
    lap = _np(lap_table)

    # --- shard: per-block graph ids, round-robin over cores
    idx_by_n = [np.where(num_nodes_arr == n)[0] for n in range(1, MAXN + 1)]
    G_ns = []
    core_ids_by_block = []  # [block][core] -> padded id array (-1 pad)
    for n in range(1, MAXN + 1):
        ids = idx_by_n[n - 1]
        per_core = [ids[c::NCORES] for c in range(NCORES)]
        G_n = max(len(x) for x in per_core) if len(ids) else 0
        G_n = (G_n + 1) & ~1  # even (fp32r matmul innermost-count rule)
        G_ns.append(G_n)
        padded = []
        for c in range(NCORES):
            a = np.full(G_n, -1, np.int64)
            a[: len(per_core[c])] = per_core[c]
            padded.append(a)
        core_ids_by_block.append(padded)

    # --- chunking: node cols per (block, chunk) with n*Gk <= PSUM_COLS
    chunks = []  # list of (n, g_start, Gk) in block order
    for bi, n in enumerate(range(1, MAXN + 1)):
        G = G_ns[bi]
        if G == 0:
            continue
        max_g = max(2, (PSUM_COLS // n) & ~1)
        s = 0
        while s < G:
            gk = min(max_g, G - s)
            chunks.append((n, s, gk))
            s += gk

    # offsets
    node_off = []
    pair_off = []
    goff = []
    N_nd = N_pr = G_c = 0
    for (n, s, gk) in chunks:
        node_off.append(N_nd)
        pair_off.append(N_pr)
        goff.append(G_c)
        N_nd += n * gk
        N_pr += (n * (n + 1) // 2) * gk
        G_c += gk

    # --- weights pack (fp32r consumed) and bias/scalar pack (fp32)
    W = _Packer()
    Bp = _Packer()

    def lin(t):
        Wt, bt = t
        return _np(Wt), _np(bt)

    wn1, bn1 = lin(p["num_net"][0]); wn2, bn2 = lin(p["num_net"][1]); wn3, bn3 = lin(p["num_net"][2])
    wg1, bg1 = lin(p["glob"][0]); wg2, bg2 = lin(p["glob"][1]); wg3, bg3 = lin(p["glob"][2])
    gins = []
    for l in range(3):
        wa, ba = lin(p["gin"][l]["A"]); wb, bb = lin(p["gin"][l]["B"])
        gins.append((wa, ba, wb, bb))
    gamma = [_np(p["gn"][l]["gamma"]) for l in range(2)]
    beta = [_np(p["gn"][l]["beta"]) for l in range(2)]
    alpha = [_np(p["gn"][l]["alpha"]) for l in range(2)]
    wf1, bf1 = lin(p["feat"][0]); wf2, bf2 = lin(p["feat"][1]); wf3, bf3 = lin(p["feat"][2])
    we1, be1 = lin(p["edge"][0]); we2, be2 = lin(p["edge"][1]); we3, be3 = lin(p["edge"][2])

    wref = {}
    wref["wn1"] = W.add(wn1); wref["wn2"] = W.add(wn2); wref["wn3"] = W.add(wn3, pad_m=128)
    wref["wg1"] = W.add(wg1, pad_m=128); wref["wg2"] = W.add(wg2, pad_m=128); wref["wg3"] = W.add(wg3, pad_m=128)
    # GIN l0: scaled Wa[:64] per block n: (1+n) * Wa64; C[n] folds lap part
    wa0, ba0 = gins[0][0], gins[0][1]
    for n in range(1, MAXN + 1):
        wref[f"wa0_{n}"] = W.add(wa0[:64, :] * (1.0 + n))
        lapn = lap[n - 1, :, :]            # [9, 9]
        lsum = lapn.sum(axis=0)            # [9]
        feats = lapn[:n, :] + lsum[None, :]  # [n, 9]
        C = feats @ wa0[64:73, :] + ba0[None, :]  # [n, 128]
        wref[f"c0_{n}"] = W.add(C)
    wref["wb0"] = W.add(gins[0][2])
    wref["wa1"] = W.add(gins[1][0]); wref["wb1"] = W.add(gins[1][2])
    wref["wa2"] = W.add(gins[2][0]); wref["wb2"] = W.add(gins[2][2])
    wref["wf1"] = W.add(wf1); wref["wf2"] = W.add(wf2); wref["wf3"] = W.add(wf3, pad_m=128)
    wref["we1h"] = W.add(we1 * 0.5)  # u = h @ (0.5 W1e) + 0.5 b1e
    wref["we2"] = W.add(we2); wref["we3"] = W.add(we3, pad_m=128)
    wref["i9"] = W.add(np.eye(MAXN, dtype=np.float32))

    bref = {}
    bref["bn1"] = Bp.add(bn1[:, None]); bref["bn2"] = Bp.add(bn2[:, None])
    bref["bg1"] = Bp.add(bg1[:, None]); bref["bg2"] = Bp.add(bg2[:, None])
    bref["bg3"] = Bp.add(bg3[:, None])
    bref["ba1"] = Bp.add(gins[1][1][:, None]); bref["ba2"] = Bp.add(gins[2][1][:, None])
    bref["bb2"] = Bp.add(gins[2][3][:, None])
    bref["bf1"] = Bp.add(bf1[:, None]); bref["bf2"] = Bp.add(bf2[:, None])
    bref["be1h"] = Bp.add(0.5 * be1[:, None])
    bref["be2"] = Bp.add(be2[:, None])
    bref["eps"] = Bp.add(np.full((128, 1), EPS, np.float32))
    for l in range(2):
        bb = gins[l][3]
        # am2 = (t1 * alpha/n) + (alpha-1)*bb  (per block n)
        bref[f"gn{l}_ao"] = Bp.add(np.stack([alpha[l] / n for n in range(1, MAXN + 1)], 1))
        bref[f"gn{l}_ab"] = Bp.add(((alpha[l] - 1.0) * bb)[:, None])
        # final: lrelu(-gamma * d + beta), d = ctil * rinv, ctil = am2 - hm
        bref[f"gn{l}_ng"] = Bp.add((-gamma[l])[:, None])
        bref[f"gn{l}_bt"] = Bp.add(beta[l][:, None])

    host = dict(
        G_ns=G_ns, core_ids_by_block=core_ids_by_block, chunks=chunks,
        node_off=node_off, pair_off=pair_off, goff=goff,
        N_nd=N_nd, N_pr=N_pr, G_c=G_c,
        WPACK=W.materialize(), BPACK=Bp.materialize(), wref=wref, bref=bref,
        bn3=bn3, bf3=bf3, be3=be3,
    )
    return host


def _emit(tc, nc, host, gv_d, pn_d, nf_d, ea_d, wp_d, bp_d):
    chunks = host["chunks"]
    node_off = host["node_off"]; pair_off = host["pair_off"]; goff = host["goff"]
    N_nd = host["N_nd"]; N_pr = host["N_pr"]; G_c = host["G_c"]
    wref = host["wref"]; bref = host["bref"]

    ctx = ExitStack()
    with ctx:
        const = ctx.enter_context(tc.tile_pool(name="const", bufs=1))
        big = ctx.enter_context(tc.tile_pool(name="big", bufs=1))
        gsp = ctx.enter_context(tc.tile_pool(name="gsp", bufs=1))
        tmp = ctx.enter_context(tc.tile_pool(name="tmp", bufs=2))
        ptmp = ctx.enter_context(tc.tile_pool(name="ptmp", bufs=2))
        psA = ctx.enter_context(tc.tile_pool(name="psA", bufs=2, space="PSUM"))
        psB = ctx.enter_context(tc.tile_pool(name="psB", bufs=2, space="PSUM"))
        psC = ctx.enter_context(tc.tile_pool(name="psC", bufs=1, space="PSUM"))

        # ---- constants
        wp = const.tile([128, host["WPACK"].shape[1]], F32R)
        nc.sync.dma_start(out=wp, in_=wp_d.bitcast(F32R))
        bp = const.tile([128, host["BPACK"].shape[1]], F32)
        nc.sync.dma_start(out=bp, in_=bp_d)

        def w(name):
            off, k, m = wref[name]
            return wp[:k, off : off + m]

        def b(name, col=0):
            off, k, m = bref[name]
            return bp[:k, off + col : off + col + 1]

        gv = const.tile([128, G_c], F32R)
        nc.sync.dma_start(out=gv, in_=gv_d.bitcast(F32R))

        def mm_evict(dst, lhsT, rhs_tile, ncols, func, bias, out_parts=128):
            """dst[:out_parts, :ncols] = func((lhsT.T @ rhs) + bias)."""
            s = 0
            while s < ncols:
                k = min(PSUM_COLS, ncols - s)
                ps = psA.tile([128, PSUM_COLS], F32, tag="ps_mm")
                nc.tensor.matmul(ps[:, :k], lhsT,
                                 rhs_tile[:, s : s + k], start=True, stop=True)
                nc.scalar.activation(out=dst[:out_parts, s : s + k],
                                     in_=ps[:out_parts, :k], func=func,
                                     bias=bias, scale=1.0, alpha=NEG)
                s += k

        # =====================================================
        # Graph-space MLPs: pred_num and glob
        # =====================================================
        s1 = gsp.tile([128, G_c], F32R)
        mm_evict(s1, w("wn1"), gv, G_c, AF.Relu, b("bn1"))
        s2 = gsp.tile([128, G_c], F32R)
        mm_evict(s2, w("wn2"), s1, G_c, AF.Relu, b("bn2"))
        pnt = gsp.tile([1, G_c], F32)
        mm_evict(pnt, w("wn3"), s2, G_c, AF.Copy, 0.0, out_parts=1)
        nc.sync.dma_start(out=pn_d, in_=pnt[:])

        g1 = gsp.tile([64, G_c], F32R)
        mm_evict(g1, w("wg1"), gv, G_c, AF.Lrelu, b("bg1"), out_parts=64)
        g2 = gsp.tile([64, G_c], F32R)
        mm_evict(g2, w("wg2"), g1, G_c, AF.Lrelu, b("bg2"), out_parts=64)
        g64 = gsp.tile([64, G_c], F32R)
        mm_evict(g64, w("wg3"), g2, G_c, AF.Identity, b("bg3"), out_parts=64)

        # =====================================================
        # GIN layer 0: z0 = (1+n)*q bcast + C[n], lrelu -> h_cur
        # =====================================================
        i9 = w("i9")
        h_cur = big.tile([128, N_nd], F32R, tag="h0")
        for ci, (n, gs, gk) in enumerate(chunks):
            no, cols = node_off[ci], n * gk
            go = goff[ci]
            ps = psA.tile([128, PSUM_COLS], F32, tag="ps_mm")
            qb = g64[:, go : go + gk].unsqueeze(1).broadcast_to([64, n, gk])
            nc.tensor.matmul(ps[:, :cols], w(f"wa0_{n}"), qb,
                             start=True, stop=False)
            cb = i9[:n, :n].unsqueeze(2).broadcast_to([n, n, gk])
            nc.tensor.matmul(ps[:, :cols], w(f"c0_{n}"), cb,
                             start=False, stop=True)
            nc.scalar.activation(out=h_cur[:, no : no + cols], in_=ps[:, :cols],
                                 func=AF.Lrelu, scale=1.0, alpha=NEG)

        ta = gsp.tile([128, G_c], F32R)

        # =====================================================
        # GIN layers
        # =====================================================
        for l in range(3):
            if l < 2:
                h_next = big.tile([128, N_nd], F32R, tag=f"h{l+1}")
                for ci, (n, gs, gk) in enumerate(chunks):
                    no, cols = node_off[ci], n * gk
                    raw = psB.tile([128, PSUM_COLS], F32, tag="raw")
                    nc.tensor.matmul(raw[:, :cols], w(f"wb{l}"),
                                     h_cur[:, no : no + cols],
                                     start=True, stop=True)
                    rawg = raw[:, :cols].rearrange("p (n g) -> p g n", n=n)
                    t1 = tmp.tile([128, PSUM_COLS], F32, tag="t1")
                    nc.vector.tensor_reduce(out=t1[:, :gk], in_=rawg,
                                            axis=AX.X, op=ALU.add)
                    am2 = tmp.tile([128, PSUM_COLS], F32, tag="am2")
                    nc.vector.tensor_scalar(out=am2[:, :gk], in0=t1[:, :gk],
                                            scalar1=b(f"gn{l}_ao", n - 1),
                                            scalar2=b(f"gn{l}_ab"),
                                            op0=ALU.mult, op1=ALU.add)
                    ctil = tmp.tile([128, PSUM_COLS], F32, tag="ctil")
                    am2b = am2[:, :gk].unsqueeze(1).broadcast_to([128, n, gk])
                    nc.vector.scalar_tensor_tensor(
                        out=ctil[:, :cols].rearrange("p (n g) -> p n g", n=n),
                        in0=am2b, scalar=1.0,
                        in1=raw[:, :cols].rearrange("p (n g) -> p n g", n=n),
                        op0=ALU.mult, op1=ALU.subtract)
                    sq = tmp.tile([128, PSUM_COLS], F32, tag="sq")
                    nc.scalar.activation(out=sq[:, :cols], in_=ctil[:, :cols],
                                         func=AF.Square, scale=1.0)
                    t2 = tmp.tile([128, PSUM_COLS], F32, tag="t2")
                    nc.vector.tensor_reduce(
                        out=t2[:, :gk],
                        in_=sq[:, :cols].rearrange("p (n g) -> p g n", n=n),
                        axis=AX.X, op=ALU.add)
                    sdev = tmp.tile([128, PSUM_COLS], F32, tag="sdev")
                    nc.scalar.activation(out=sdev[:, :gk], in_=t2[:, :gk],
                                         func=AF.Sqrt, scale=1.0 / n,
                                         bias=b("eps"))
                    rinv = tmp.tile([128, PSUM_COLS], F32, tag="rinv")
                    scr = tmp.tile([128, PSUM_COLS], F32, tag="scr")
                    nc.vector.reciprocal_approx_accurate(out=rinv[:, :gk],
                                                         in_=sdev[:, :gk],
                                                         scratch=scr[:, :gk])
                    d = tmp.tile([128, PSUM_COLS], F32, tag="d")
                    rb = rinv[:, :gk].unsqueeze(1).broadcast_to([128, n, gk])
                    nc.gpsimd.tensor_tensor(
                        out=d[:, :cols].rearrange("p (n g) -> p n g", n=n),
                        in0=ctil[:, :cols].rearrange("p (n g) -> p n g", n=n),
                        in1=rb, op=ALU.mult)
                    nc.scalar.activation(out=h_next[:, no : no + cols],
                                         in_=d[:, :cols], func=AF.Lrelu,
                                         bias=b(f"gn{l}_bt"),
                                         scale=b(f"gn{l}_ng"), alpha=NEG)
                h_cur = h_next
                # aggregation + Wa_{l+1} + lrelu
                for ci, (n, gs, gk) in enumerate(chunks):
                    no, cols = node_off[ci], n * gk
                    go = goff[ci]
                    hg = h_cur[:, no : no + cols].bitcast(F32).rearrange(
                        "p (n g) -> p g n", n=n)
                    with nc.allow_low_precision(reason="f32r rounding"):
                        nc.vector.tensor_reduce(out=ta[:, go : go + gk],
                                                in_=hg, axis=AX.X, op=ALU.add)
                a_out = big.tile([128, N_nd], F32R, tag=f"a{l+1}")
                for ci, (n, gs, gk) in enumerate(chunks):
                    no, cols = node_off[ci], n * gk
                    go = goff[ci]
                    ps = psA.tile([128, PSUM_COLS], F32, tag="ps_mm")
                    wa = w(f"wa{l+1}")
                    nc.tensor.matmul(ps[:, :cols], wa,
                                     h_cur[:, no : no + cols],
                                     start=True, stop=False)
                    tab = ta[:, go : go + gk].unsqueeze(1).broadcast_to(
                        [128, n, gk])
                    nc.tensor.matmul(ps[:, :cols], wa, tab,
                                     start=False, stop=True)
                    nc.scalar.activation(out=a_out[:, no : no + cols],
                                         in_=ps[:, :cols], func=AF.Lrelu,
                                         bias=b(f"ba{l+1}"), scale=1.0,
                                         alpha=NEG)
                h_cur = a_out
            else:
                h3 = big.tile([128, N_nd], F32R, tag="h3")
                mm_evict(h3, w("wb2"), h_cur, N_nd, AF.Identity, b("bb2"))
                h_cur = h3

        h3 = h_cur

        # =====================================================
        # feat MLP -> nf (bias bf3 added on host), chunked
        # =====================================================
        s = 0
        while s < N_nd:
            k = min(PSUM_COLS, N_nd - s)
            ps = psA.tile([128, PSUM_COLS], F32, tag="ps_mm")
            nc.tensor.matmul(ps[:, :k], w("wf1"), h3[:, s : s + k],
                             start=True, stop=True)
            f1 = tmp.tile([128, PSUM_COLS], F32R, tag="f1")
            nc.scalar.activation(out=f1[:, :k], in_=ps[:, :k], func=AF.Lrelu,
                                 bias=b("bf1"), scale=1.0, alpha=NEG)
            ps2 = psB.tile([128, PSUM_COLS], F32, tag="raw")
            nc.tensor.matmul(ps2[:, :k], w("wf2"), f1[:, :k],
                             start=True, stop=True)
            f2 = tmp.tile([128, PSUM_COLS], F32R, tag="f2")
            nc.scalar.activation(out=f2[:, :k], in_=ps2[:, :k], func=AF.Lrelu,
                                 bias=b("bf2"), scale=1.0, alpha=NEG)
            ps3 = psC.tile([128, PSUM_COLS], F32, tag="ps_out")
            nc.tensor.matmul(ps3[:, :k], w("wf3"), f2[:, :k],
                             start=True, stop=True)
            nfs = tmp.tile([4, PSUM_COLS], F32, tag="nfs")
            if (s // PSUM_COLS) % 2 == 0:
                nc.vector.tensor_copy(out=nfs[:, :k], in_=ps3[:4, :k])
            else:
                nc.scalar.activation(out=nfs[:, :k], in_=ps3[:4, :k],
                                     func=AF.Copy, bias=0.0, scale=1.0)
            nc.sync.dma_start(out=nf_d[:, s : s + k], in_=nfs[:, :k])
            s += k

        # =====================================================
        # edge stage, per (block, chunk)
        # =====================================================
        for ci, (n, gs, gk) in enumerate(chunks):
            no, cols = node_off[ci], n * gk
            po = pair_off[ci]
            npair = (n * (n + 1) // 2) * gk
            # u = 0.5*(h3 @ W1e + b1e) for this chunk
            psu = psA.tile([128, PSUM_COLS], F32, tag="ps_mm")
            nc.tensor.matmul(psu[:, :cols], w("we1h"), h3[:, no : no + cols],
                             start=True, stop=True)
            u = ptmp.tile([128, PSUM_COLS], F32, tag="u")
            nc.scalar.activation(out=u[:, :cols], in_=psu[:, :cols],
                                 func=AF.Identity, bias=b("be1h"), scale=1.0)
            # z1 runs: z1[:, run_i] = u_i (bcast) + u_[i:]
            z1_max = max((cn * (cn + 1) // 2) * cgk for (cn, _, cgk) in chunks)
            z1 = ptmp.tile([128, z1_max], F32, tag="z1")
            run_off = 0
            for i in range(n):
                rl = (n - i) * gk
                ui = u[:, i * gk : (i + 1) * gk]
                uib = ui.unsqueeze(1).broadcast_to([128, n - i, gk])
                uj = u[:, i * gk : n * gk].rearrange("p (m g) -> p m g",
                                                     m=n - i)
                nc.vector.scalar_tensor_tensor(
                    out=z1[:, run_off : run_off + rl].rearrange(
                        "p (m g) -> p m g", m=n - i),
                    in0=uib, scalar=1.0, in1=uj, op0=ALU.mult, op1=ALU.add)
                run_off += rl
            # a1 = lrelu(z1); mm2; a2; mm3; dma
            for s2 in range(0, npair, PSUM_COLS):
                k2 = min(PSUM_COLS, npair - s2)
                a1 = ptmp.tile([128, PSUM_COLS], F32R, tag="a1")
                nc.scalar.activation(out=a1[:, :k2], in_=z1[:, s2 : s2 + k2],
                                     func=AF.Lrelu, scale=1.0, alpha=NEG)
                ps = psB.tile([128, PSUM_COLS], F32, tag="raw")
                nc.tensor.matmul(ps[:, :k2], w("we2"), a1[:, :k2],
                                 start=True, stop=True)
                a2 = ptmp.tile([128, PSUM_COLS], F32R, tag="a2")
                nc.scalar.activation(out=a2[:, :k2], in_=ps[:, :k2],
                                     func=AF.Lrelu, bias=b("be2"), scale=1.0,
                                     alpha=NEG)
                ps3 = psC.tile([128, PSUM_COLS], F32, tag="ps_out")
                nc.tensor.matmul(ps3[:, :k2], w("we3"), a2[:, :k2],
                                 start=True, stop=True)
                eas = ptmp.tile([5, PSUM_COLS], F32, tag="eas")
                if (s2 // PSUM_COLS) % 2 == 0:
                    nc.vector.tensor_copy(out=eas[:, :k2], in_=ps3[:5, :k2])
                else:
                    nc.scalar.activation(out=eas[:, :k2], in_=ps3[:5, :k2],
                                         func=AF.Copy, bias=0.0, scale=1.0)
                nc.sync.dma_start(out=ea_d[:, po + s2 : po + s2 + k2],
                                  in_=eas[:, :k2])


def _build_program(host):
    nc = bacc.Bacc("TRN2", debug=False)
    G_c, N_nd, N_pr = host["G_c"], host["N_nd"], host["N_pr"]
    gv_d = nc.dram_tensor("gv", [128, G_c], F32, kind="ExternalInput").ap()
    wp_d = nc.inline_tensor(host["WPACK"], name="wpack").ap()
    bp_d = nc.inline_tensor(host["BPACK"], name="bpack").ap()
    pn_d = nc.dram_tensor("pn", [1, G_c], F32, kind="ExternalOutput").ap()
    nf_d = nc.dram_tensor("nf", [4, N_nd], F32, kind="ExternalOutput").ap()
    ea_d = nc.dram_tensor("ea", [5, N_pr], F32, kind="ExternalOutput").ap()
    with tile.TileContext(nc) as tc:
        _emit(tc, nc, host, gv_d, pn_d, nf_d, ea_d, wp_d, bp_d)
    nc.compile()
    return nc


_CACHE = {}


def _get_program(host):
    import hashlib
    hsh = hashlib.sha1(host["WPACK"].tobytes() + host["BPACK"].tobytes()
                       ).hexdigest()
    key = (tuple(host["chunks"]), hsh, "v1")
    if key not in _CACHE:
        _CACHE[key] = _build_program(host)
    return _CACHE[key]


def _make_in_maps(host, gvec):
    chunks = host["chunks"]; goff = host["goff"]; G_c = host["G_c"]
    in_maps = []
    col_gid = np.full((NCORES, G_c), -1, np.int64)
    for c in range(NCORES):
        gvT = np.zeros((128, G_c), np.float32)
        for ci, (n, gs, gk) in enumerate(chunks):
            bi = n - 1
            ids = host["core_ids_by_block"][bi][c][gs : gs + gk]
            go = goff[ci]
            col_gid[c, go : go + gk] = ids
            valid = ids >= 0
            if valid.any():
                gvT[:, go : go + gk][:, valid] = gvec[ids[valid]].T
        in_maps.append({"gv": gvT})
    return in_maps, col_gid


def _scatter_outputs(host, results, col_gid, B):
    chunks = host["chunks"]
    node_off = host["node_off"]; pair_off = host["pair_off"]; goff = host["goff"]
    N_nd, N_pr = host["N_nd"], host["N_pr"]
    nd_slot = np.empty(N_nd, np.int64)
    nd_node = np.empty(N_nd, np.int64)
    pr_slot = np.empty(N_pr, np.int64)
    pr_i = np.empty(N_pr, np.int64)
    pr_j = np.empty(N_pr, np.int64)
    for ci, (n, gs, gk) in enumerate(chunks):
        no, po, go = node_off[ci], pair_off[ci], goff[ci]
        for i in range(n):
            sl = slice(no + i * gk, no + (i + 1) * gk)
            nd_slot[sl] = np.arange(go, go + gk)
            nd_node[sl] = i
        off = po
        for i in range(n):
            for j in range(i, n):
                sl = slice(off, off + gk)
                pr_slot[sl] = np.arange(go, go + gk)
                pr_i[sl] = i
                pr_j[sl] = j
                off += gk

    bn3, bf3, be3 = host["bn3"], host["bf3"], host["be3"]
    pred_num = np.zeros(B, np.float32)
    node_feats = np.zeros((B, MAXN, 4), np.float32)
    edge_attr = np.zeros((B, MAXN, MAXN, 5), np.float32)

    for c in range(NCORES):
        r = results[c]
        gid = col_gid[c]
        gv_valid = gid >= 0
        pred_num[gid[gv_valid]] = r["pn"][0, gv_valid] + bn3[0]

        nd_gid = gid[nd_slot]
        v = nd_gid >= 0
        node_feats[nd_gid[v], nd_node[v], :] = r["nf"][:, v].T + bf3[None, :]

        pr_gid = gid[pr_slot]
        v = pr_gid >= 0
        vals = r["ea"][:, v].T + be3[None, :]
        edge_attr[pr_gid[v], pr_i[v], pr_j[v], :] = vals
        edge_attr[pr_gid[v], pr_j[v], pr_i[v], :] = vals

    return node_feats, edge_attr, pred_num


def kernel(global_vec, num_nodes, lap_table, params):
    gvec = np.asarray(global_vec, np.float32)
    nn_arr = np.asarray(num_nodes).astype(np.int64)
    B = gvec.shape[0]

    host = _prep_host(params, lap_table, nn_arr)
    nc = _get_program(host)
    in_maps, col_gid = _make_in_maps(host, gvec)
    res = run_bass_kernel_spmd(nc, in_maps, list(range(NCORES)))
    return _scatter_outputs(host, res.results, col_gid, B)
